# revision 41
# baseline (speedup 1.0000x reference)
"""DG-block (dual graph-conv) Trainium2 kernel — nn_DG_Block.

Reference per batch item b (B=8, C=128, N=2000, K=9):
  idx1 = top9(knn keys on features_b); idx2 = top9(... motion_b)
  gf_i = graph_feature(features_b, idx_i) -> [2C, N, 9]
  f_i  = conv_bn_relu(1x3 stride 3) -> conv_bn_relu(1x3) on gf_i
  out_b = f1 + delta * f2        [C, N, 1]
BatchNorm pools over the WHOLE batch -> stats are all-reduced across cores.

Sharding: one batch item per NeuronCore (8 cores); conv/BN params replicated;
two [128,4] AllReduces reproduce the exact batch statistics.

Algebra (per branch; w1 [C,2C,1,3] split A_d = w1[:,:C,0,d], B_d = w1[:,C:,0,d]):
  conv1[o,n,t] = (P x_n)[o] - sum_d (B_d x_{idx[n,3t+d]})[o],  P = sum_d A_d+B_d
  (conv biases dropped: BN mean-subtraction cancels them exactly)
  knn rank key: <x_i, x_j> - |x_j|^2/2  (monotone per-row transform of the
  reference's 2<x_i,x_j> - |x_i|^2 - |x_j|^2; rank-1 is always the point
  itself -> hardcoded; ranks 2..9 via DVE max8 + max_index with the diagonal
  masked to -1e30)

Device pipeline per core:
  kNN    : pd chunk [128,2048] = X_chunk^T X on PE (fp32) + (-|x_j|^2/2) via
           SWDGE broadcast-accumulate DMA; max8 + max_index on DVE.
  tables : ytab[n, d*C:..] = (-B_d x_n)^T and Z^T rows (PE, ACT copy, DMA).
  conv1  : per (chunk,t): g = Z^T rows + 3 gather-accumulate taps from ytab
           (indirect SWDGE, compute_op=add), PE-transpose -> o1 psum;
           ACT Copy/Square with accum_out -> stats; AllReduce; ACT Relu-affine.
  conv2  : 3 accumulated matmuls; stats; AllReduce; final Relu-affines,
           f1 + delta*f2 on DVE, DMA out.
"""

import numpy as np

import concourse.bacc as bacc
import concourse.bass as bass
import concourse.mybir as mybir
import concourse.tile as tile
import concourse.bass_utils as bass_utils
from concourse.masks import make_identity

F32 = mybir.dt.float32
F16 = mybir.dt.float16
U32 = mybir.dt.uint32
I16 = mybir.dt.int16
AF = mybir.ActivationFunctionType
ALU = mybir.AluOpType

B = 8
C = 128
N = 2000
EPS = 1e-5
NEG_BIG = -1.0e30

CHUNKS = [(i * 128, min(128, N - i * 128)) for i in range((N + 127) // 128)]
NCH = len(CHUNKS)  # 16
# pd column tiles, 512-aligned so the diagonal block never straddles tiles
JT = [(j * 512, min(512, N - j * 512)) for j in range(4)]


def build_kernel(delta_nonneg: bool):
    nc = bacc.Bacc(
        "TRN2",
        target_bir_lowering=False,
        debug=False,
        enable_asserts=False,
        num_devices=B,
        num_swdge_queues=4,
    )

    feat_in = nc.dram_tensor("feat", [C, N], F32, kind="ExternalInput").ap()
    mot_in = nc.dram_tensor("mot", [C, N], F32, kind="ExternalInput").ap()
    wb = {}
    for br in (1, 2):
        wb[br] = {
            "pt": nc.dram_tensor(f"pt{br}", [C, C], F32, kind="ExternalInput").ap(),
            "nbt": nc.dram_tensor(f"nbt{br}", [C, 3 * C], F32, kind="ExternalInput").ap(),
            "w2t": nc.dram_tensor(f"w2t{br}", [C, 3 * C], F32, kind="ExternalInput").ap(),
            "bn": nc.dram_tensor(f"bn{br}", [C, 4], F32, kind="ExternalInput").ap(),
        }
    delta_in = nc.dram_tensor("delta", [1, 1], F32, kind="ExternalInput").ap()
    out_t = nc.dram_tensor("out", [C, N], F32, kind="ExternalOutput").ap()

    with tile.TileContext(nc) as tc:
        _emit(nc, tc, feat_in, mot_in, wb, delta_in, out_t, delta_nonneg)
    nc.compile()
    return nc


def _emit(nc, tc, feat_in, mot_in, wb, delta_in, out_t, delta_nonneg):
    import contextlib

    ctx = contextlib.ExitStack()
    with ctx:
        sb = ctx.enter_context(tc.tile_pool(name="sb", bufs=1))
        pd_ps = ctx.enter_context(tc.tile_pool(name="pd_ps", bufs=2, space="PSUM"))
        st_ps = ctx.enter_context(tc.tile_pool(name="st_ps", bufs=2, space="PSUM"))
        o1_ps = ctx.enter_context(tc.tile_pool(name="o1_ps", bufs=2, space="PSUM"))
        dr = ctx.enter_context(tc.tile_pool(name="dr", bufs=1, space="DRAM"))

        # ---------------- persistent on-chip data ----------------
        x = sb.tile([C, N], F32, name="x")
        nc.sync.dma_start(out=x[:], in_=feat_in)
        m = sb.tile([C, N], F32, name="m")
        nc.sync.dma_start(out=m[:], in_=mot_in)

        ident = sb.tile([C, C], F32, name="ident")
        make_identity(nc, ident[:])
        ineg = sb.tile([C, C], F32, name="ineg")
        nc.scalar.activation(out=ineg[:], in_=ident[:], func=AF.Copy, scale=NEG_BIG)
        ones1 = sb.tile([1, C], F32, name="ones1")
        nc.vector.memset(ones1[:], 1.0)
        neghalfc = sb.tile([C, 1], F32, name="neghalfc")
        nc.vector.memset(neghalfc[:], -0.5)

        w = {}
        for br in (1, 2):
            pt = sb.tile([C, C], F32, name=f"pt{br}")
            nc.sync.dma_start(out=pt[:], in_=wb[br]["pt"])
            nbt = sb.tile([C, 3 * C], F32, name=f"nbt{br}")
            nc.sync.dma_start(out=nbt[:], in_=wb[br]["nbt"])
            w2t = sb.tile([C, 3 * C], F32, name=f"w2t{br}")
            nc.sync.dma_start(out=w2t[:], in_=wb[br]["w2t"])
            bn = sb.tile([C, 4], F32, name=f"bn{br}")
            nc.sync.dma_start(out=bn[:], in_=wb[br]["bn"])
            w[br] = dict(pt=pt, nbt=nbt, w2t=w2t, bn=bn)

        delta_sb = sb.tile([1, 1], F32, name="delta_sb")
        nc.sync.dma_start(out=delta_sb[:], in_=delta_in)
        # broadcast delta to a [C,1] column via K=1 matmul
        dps = st_ps.tile([C, 8], F32, name="dps", tag="stage")
        nc.tensor.matmul(
            out=dps[:, 0:1], lhsT=ones1[:], rhs=delta_sb[0:1, 0:1], start=True, stop=True
        )
        dcol = sb.tile([C, 1], F32, name="dcol")
        nc.scalar.activation(out=dcol[:], in_=dps[:, 0:1], func=AF.Copy)

        ytab = {br: dr.tile([N, 3 * C], F32, name=f"ytab{br}") for br in (1, 2)}
        idx8 = {s: sb.tile([C, NCH * 8], U32, name=f"idx8_{s}") for s in (1, 2)}

        # --- batched-gather offset tables ---
        # ytab viewed as [3N, C]: row 3n+d = (-B_d x_n)^T, so tap (n, j) is
        # row 3*idx[n,j] + (j mod 3); one 1024-index dma_gather per chunk
        # replaces 8 single-tap indirect DMAs (SWDGE fixed overhead
        # ~1us/instruction dominates the gather wall). Self tap (j=0) is
        # contiguous ytab rows -> plain DMA. idxw holds int16 indices in the
        # 16-partition-wrapped layout dma_gather expects (chunk index i at
        # [i%16, i//16], replicated to all 8 GpSimd core groups), built from
        # idx9 via a u16-bitcast DRAM bounce (j-major flat == wrapped layout
        # re-read with r fastest).
        pat8 = sb.tile([C, 8], U32, name="pat8")
        for j in range(1, 9):
            nc.vector.memset(pat8[:, j - 1 : j], j % 3)
        idx9 = {s: sb.tile([C, NCH * 8], U32, name=f"idx9_{s}") for s in (1, 2)}
        idxw = {s: sb.tile([C, NCH * 64], I16, name=f"idxw_{s}") for s in (1, 2)}

        def build_idx9(which):
            v = idx9[which][:, 0 : NCH * 8].rearrange("p (ci j) -> p ci j", j=8)
            i8 = idx8[which][:, 0 : NCH * 8].rearrange("p (ci j) -> p ci j", j=8)
            nc.vector.tensor_scalar_mul(v, i8, 3)
            p8 = pat8[:, 0:8].rearrange("p (x j) -> p x j", x=1)
            p8b, _ = bass.broadcast_tensor_aps(p8, v)
            nc.vector.tensor_tensor(out=v, in0=v, in1=p8b, op=ALU.add)
            # last chunk has 80 valid points; the tail rows hold garbage from
            # max_index — clamp all indices into the valid row range so the
            # gather ucode never sees an OOB or negative-as-int16 index
            nc.vector.tensor_scalar_min(idx9[which][:], idx9[which][:], 3 * N - 1)
            # wrap for the ucode via X-bar transpose + DVE column shuffle
            # (every DMA keeps contiguous runs; a naive transposing DMA
            # explodes into 2-byte descriptors)
            loc = sb.tile([C, C], I16, name=f"loc_{which}")
            lo16 = idx9[which][:, 0 : NCH * 8].bitcast(I16).rearrange(
                "p (c two) -> p c two", two=2
            )[:, :, 0]
            nc.vector.tensor_tensor(out=loc[:], in0=lo16, in1=lo16, op=ALU.bypass)
            tt = sb.tile([C, C], I16, name=f"tt_{which}")
            nc.sync.dma_start_transpose(out=tt[:], in_=loc[:])
            tt2 = sb.tile([C, C], I16, name=f"tt2_{which}")
            dstv = tt2[:, 0:C].rearrange("q (rr ph) -> q rr ph", rr=16)
            srcv = tt[:, 0:C].rearrange("q (ph rr) -> q ph rr", ph=8).rearrange(
                "q ph rr -> q rr ph"
            )
            nc.vector.tensor_tensor(out=dstv, in0=srcv, in1=srcv, op=ALU.bypass)
            d2 = dr.tile([16, NCH * 64], I16, name=f"ibounce_{which}")
            d2v = d2[:, :].rearrange("rr (cj ph) -> cj rr ph", cj=128, ph=8)
            nc.sync.dma_start(
                out=d2v,
                in_=tt2[:, 0:C].rearrange("q (rr ph) -> q rr ph", rr=16),
            )
            # the gather ucode's worker core pair reads its own 16-partition
            # copy of the wrapped indices -> replicate into all 8 groups
            for k in range(8):
                nc.sync.dma_start(
                    out=idxw[which][16 * k : 16 * k + 16, :], in_=d2[:, :]
                )

        # ---------------- kNN (per-chunk emitters) ----------------
        # pd stays fp32 end-to-end: top-8 rank gaps (~0.6) vs ~15% per-point
        # error per neighbor swap mean any quantization blows the tolerance.
        knn_aux = {}

        def knn_prep(src, which):
            xsq = sb.tile([C, N], F32, name=f"xsq_{which}", tag="xsq", bufs=1)
            nc.scalar.activation(out=xsq[:], in_=src[:], func=AF.Square)
            sqrow = sb.tile([1, N], F32, name=f"sqrow_{which}", tag="sqrow", bufs=1)
            for j0, jn in JT:
                sqps = st_ps.tile([1, 512], F32, name=f"sqps_{which}_{j0}", tag="stage")
                nc.tensor.matmul(
                    out=sqps[0:1, :jn],
                    lhsT=neghalfc[:],
                    rhs=xsq[:, j0 : j0 + jn],
                    start=True,
                    stop=True,
                )
                nc.scalar.activation(
                    out=sqrow[0:1, j0 : j0 + jn], in_=sqps[0:1, :jn], func=AF.Copy
                )
            negsqh = sb.tile([C, N], F32, name=f"negsqh_{which}", tag="negsqh", bufs=1)
            for j0, jn in JT:
                nps = st_ps.tile([C, 512], F32, name=f"nps_{which}_{j0}", tag="stage")
                nc.tensor.matmul(
                    out=nps[:, :jn],
                    lhsT=ones1[:],
                    rhs=sqrow[0:1, j0 : j0 + jn],
                    start=True,
                    stop=True,
                )
                nc.scalar.activation(
                    out=negsqh[:, j0 : j0 + jn], in_=nps[:, :jn], func=AF.Copy
                )
            knn_aux[which] = negsqh

        def knn_chunk(src, which, ci, use_pool=False):
            c0, cn = CHUNKS[ci]
            eng = nc.gpsimd if use_pool else nc.vector
            negsqh = knn_aux[which]
            pdt = sb.tile([C, N], F32, name=f"pd_{which}_{ci}", tag="pd", bufs=2)
            for half in range(2):
                pps = pd_ps.tile(
                    [C, 1024], F32, name=f"pps_{which}_{ci}_{half}", tag="pdps"
                )
                for sub in range(2):
                    j0, jn = JT[half * 2 + sub]
                    nc.tensor.matmul(
                        out=pps[:cn, sub * 512 : sub * 512 + jn],
                        lhsT=src[:, c0 : c0 + cn],
                        rhs=src[:, j0 : j0 + jn],
                        start=True,
                        stop=True,
                    )
                w0 = JT[half * 2][1] + JT[half * 2 + 1][1]
                nc.scalar.activation(
                    out=pdt[:cn, half * 1024 : half * 1024 + w0],
                    in_=pps[:cn, 0:w0],
                    func=AF.Copy,
                )
            eng.tensor_tensor(
                out=pdt[:cn, :], in0=pdt[:cn, :], in1=negsqh[:cn, :], op=ALU.add
            )
            eng.tensor_tensor(
                out=pdt[:cn, c0 : c0 + cn],
                in0=pdt[:cn, c0 : c0 + cn],
                in1=ineg[:cn, :cn],
                op=ALU.add,
            )
            vals8 = sb.tile([C, 8], F32, name=f"v8_{which}_{ci}", tag="v8", bufs=2)
            nc.vector.max(out=vals8[:cn], in_=pdt[:cn, :])
            nc.vector.max_index(
                out=idx8[which][:cn, ci * 8 : ci * 8 + 8],
                in_max=vals8[:cn],
                in_values=pdt[:cn, :],
            )

        # ---------------- Y/Z tables ----------------
        ztiles = {}

        def tables_init(br):
            ztiles[br] = []

        def tables_chunk(br, ci):
            c0, cn = CHUNKS[ci]
            if True:
                yps = st_ps.tile([C, 384], F32, name=f"yps_{br}_{ci}", tag="stage")
                nc.tensor.matmul(
                    out=yps[:cn, :],
                    lhsT=x[:, c0 : c0 + cn],
                    rhs=w[br]["nbt"][:],
                    start=True,
                    stop=True,
                )
                yst = sb.tile([C, 384], F32, name=f"yst_{br}_{ci}", tag="yst", bufs=3)
                nc.scalar.activation(out=yst[:cn, :], in_=yps[:cn, :], func=AF.Copy)
                nc.sync.dma_start(out=ytab[br][c0 : c0 + cn, :], in_=yst[:cn, :])

                zps = st_ps.tile([C, 128], F32, name=f"zps_{br}_{ci}", tag="stage")
                nc.tensor.matmul(
                    out=zps[:cn, :],
                    lhsT=x[:, c0 : c0 + cn],
                    rhs=w[br]["pt"][:],
                    start=True,
                    stop=True,
                )
                zt = sb.tile([C, C], F32, name=f"zt_{br}_{ci}", tag=f"zt{br}", bufs=NCH)
                nc.scalar.activation(out=zt[:cn, :], in_=zps[:cn, :], func=AF.Copy)
                ztiles[br].append(zt)

        # ---------------- conv1 + stats (per-chunk) ----------------
        o1_tiles = {}
        stats1 = {}

        def conv1_init(br):
            stats1[br] = (
                sb.tile([C, NCH], F32, name=f"s1c_{br}"),
                sb.tile([C, NCH], F32, name=f"s2c_{br}"),
            )
            o1_tiles[br] = []

        def conv1_chunk(br, which, ci):
            s1, s2 = stats1[br]
            c0, cn = CHUNKS[ci]
            ytab3 = ytab[br][:, :].rearrange("n (d c) -> (n d) c", d=3)
            g9 = sb.tile([C, 9 * C], F32, name=f"g9_{br}_{ci}", tag="g9", bufs=6)
            # self tap (j=0, d=0): contiguous ytab rows -> direct DMA
            nc.sync.dma_start(out=g9[:cn, 0:C], in_=ytab[br][c0 : c0 + cn, 0:C])
            nc.gpsimd.dma_gather(
                out_ap=g9[:, C : 9 * C].rearrange("p (q e) -> p q e", q=8),
                in_ap=ytab3,
                idxs_ap=idxw[which][:, ci * 64 : ci * 64 + 64],
                num_idxs=8 * 128,
                num_idxs_reg=8 * 128,
                elem_size=C,
                queue_num=ci % 4,
            )
            # 9 taps -> 3 conv windows (sum of 3 taps each) + broadcast z
            g3 = sb.tile([C, 3 * C], F32, name=f"g3_{br}_{ci}", tag="g3", bufs=3)
            g9v = g9[:, 0 : 9 * C].rearrange("p (t d c) -> p t d c", t=3, d=3)
            g3v = g3[:, 0 : 3 * C].rearrange("p (t c) -> p t c", t=3)
            nc.vector.tensor_tensor(
                out=g3v[:cn], in0=g9v[:cn, :, 0, :], in1=g9v[:cn, :, 1, :], op=ALU.add
            )
            nc.vector.tensor_tensor(
                out=g3v[:cn], in0=g3v[:cn], in1=g9v[:cn, :, 2, :], op=ALU.add
            )
            zb = ztiles[br][ci][:, 0:C].rearrange("p (x c) -> p x c", x=1)[:cn]
            zbb, _ = bass.broadcast_tensor_aps(zb, g3v[:cn])
            nc.vector.tensor_tensor(out=g3v[:cn], in0=g3v[:cn], in1=zbb, op=ALU.add)
            ops = o1_ps.tile([C, 384], F32, name=f"o1ps_{br}_{ci}", tag="o1")
            for t in range(3):
                nc.tensor.matmul(
                    out=ops[:, t * C : t * C + cn],
                    lhsT=g3[:cn, t * C : t * C + C],
                    rhs=ident[:cn, :cn],
                    is_transpose=True,
                    start=True,
                    stop=True,
                    skip_group_check=True,
                )
            # stats + store o1 (strided [C,3,cn] views skip padding columns)
            src_ap = ops[:, 0:384].rearrange("p (t n) -> p t n", t=3)[:, :, :cn]
            ot = sb.tile([C, 384], F32, name=f"o1_{br}_{ci}", tag=f"o1{br}", bufs=NCH)
            dst_ap = ot[:, 0:384].rearrange("p (t n) -> p t n", t=3)[:, :, :cn]
            nc.scalar.activation(
                out=dst_ap, in_=src_ap, func=AF.Copy, accum_out=s1[:, ci : ci + 1]
            )
            osq = sb.tile([C, 384], F32, name=f"o1sq_{br}_{ci}", tag="o1sq", bufs=2)
            sq_ap = osq[:, 0:384].rearrange("p (t n) -> p t n", t=3)[:, :, :cn]
            nc.scalar.activation(
                out=sq_ap, in_=src_ap, func=AF.Square, accum_out=s2[:, ci : ci + 1]
            )
            o1_tiles[br].append(ot)

        # ---------------- allreduce + affine ----------------
        def stats_cols(stats, arq, col):
            s1, s2 = stats
            nc.vector.reduce_sum(
                out=arq[:, col : col + 1], in_=s1[:], axis=mybir.AxisListType.X
            )
            nc.vector.reduce_sum(
                out=arq[:, col + 1 : col + 2], in_=s2[:], axis=mybir.AxisListType.X
            )

        def allreduce(arq, k, name):
            ar_in = dr.tile([C, k], F32, name=f"arin{name}")
            ar_out = dr.tile([C, k], F32, name=f"arout{name}", addr_space="Shared")
            nc.sync.dma_start(out=ar_in[:], in_=arq[:])
            nc.gpsimd.collective_compute(
                "AllReduce",
                ALU.add,
                replica_groups=[list(range(B))],
                ins=[ar_in[:].opt()],
                outs=[ar_out[:].opt()],
            )
            art = sb.tile([C, k], F32, name=f"art{name}")
            nc.sync.dma_start(out=art[:], in_=ar_out[:])
            return art

        def affine_from(art, col, m_count, br, bn_cols, name):
            inv_m = 1.0 / float(m_count)
            gcol = w[br]["bn"][:, bn_cols[0] : bn_cols[0] + 1]
            bcol = w[br]["bn"][:, bn_cols[1] : bn_cols[1] + 1]
            mean = sb.tile([C, 1], F32, name=f"mean{name}")
            nc.vector.tensor_scalar_mul(mean[:], art[:, col : col + 1], inv_m)
            ey2 = sb.tile([C, 1], F32, name=f"ey2{name}")
            nc.vector.tensor_scalar_mul(ey2[:], art[:, col + 1 : col + 2], inv_m)
            var = sb.tile([C, 1], F32, name=f"var{name}")
            nc.vector.tensor_tensor(out=var[:], in0=mean[:], in1=mean[:], op=ALU.mult)
            nc.vector.tensor_tensor(out=var[:], in0=ey2[:], in1=var[:], op=ALU.subtract)
            nc.vector.tensor_scalar_add(var[:], var[:], EPS)
            rv = sb.tile([C, 1], F32, name=f"rv{name}")
            nc.vector.reciprocal(rv[:], var[:])
            rstd = sb.tile([C, 1], F32, name=f"rstd{name}")
            nc.scalar.activation(out=rstd[:], in_=rv[:], func=AF.Sqrt)
            a_col = sb.tile([C, 1], F32, name=f"acol{name}")
            nc.vector.tensor_tensor(out=a_col[:], in0=gcol, in1=rstd[:], op=ALU.mult)
            c_col = sb.tile([C, 1], F32, name=f"ccol{name}")
            nc.vector.tensor_tensor(out=c_col[:], in0=mean[:], in1=a_col[:], op=ALU.mult)
            nc.vector.tensor_tensor(out=c_col[:], in0=bcol, in1=c_col[:], op=ALU.subtract)
            return (a_col, c_col)

        # ---------------- conv2 + stats (per-chunk) ----------------
        o2_tiles = {}
        stats2 = {}

        def conv2_init(br):
            stats2[br] = (
                sb.tile([C, NCH], F32, name=f"s1d_{br}"),
                sb.tile([C, NCH], F32, name=f"s2d_{br}"),
            )
            o2_tiles[br] = []

        def conv2_chunk(br, aff, ci):
            a_col, c_col = aff
            s1, s2 = stats2[br]
            c0, cn = CHUNKS[ci]
            ot = o1_tiles[br][ci]
            o1r_ap = ot[:, 0:384].rearrange("p (t n) -> p t n", t=3)[:, :, :cn]
            nc.scalar.activation(
                out=o1r_ap, in_=o1r_ap, func=AF.Relu, scale=a_col[:], bias=c_col[:]
            )
            o2ps = st_ps.tile([C, 128], F32, name=f"o2ps_{br}_{ci}", tag="stage")
            for d in range(3):
                nc.tensor.matmul(
                    out=o2ps[:, :cn],
                    lhsT=w[br]["w2t"][:, d * C : (d + 1) * C],
                    rhs=ot[:, d * C : d * C + cn],
                    start=(d == 0),
                    stop=(d == 2),
                )
            o2 = sb.tile([C, C], F32, name=f"o2_{br}_{ci}", tag=f"o2{br}", bufs=NCH)
            nc.scalar.activation(
                out=o2[:, :cn], in_=o2ps[:, :cn], func=AF.Copy,
                accum_out=s1[:, ci : ci + 1],
            )
            osq = sb.tile([C, C], F32, name=f"o2sq_{br}_{ci}", tag="o2sq", bufs=2)
            nc.scalar.activation(
                out=osq[:, :cn], in_=o2ps[:, :cn], func=AF.Square,
                accum_out=s2[:, ci : ci + 1],
            )
            o2_tiles[br].append(o2)

        # ---------------- emit: chunk-interleaved schedule ----------------
        knn_prep(x, 1)
        for ci in range(NCH):
            knn_chunk(x, 1, ci, use_pool=True)
        build_idx9(1)
        # both tables depend only on x + weights; emitting them here lets
        # their PE/ACT/DMA work drain during late kNN so neither branch's
        # gathers ever wait on ytab
        tables_init(1)
        for ci in range(NCH):
            tables_chunk(1, ci)
        knn_prep(m, 2)
        tables_init(2)
        conv1_init(1)
        # knn on motion, branch-1 conv1, and the branch-2 tables are
        # independent chains: alternate them per chunk so every engine stays
        # fed without any queue-order head-of-line blocking
        for ci in range(NCH):
            knn_chunk(m, 2, ci)
            conv1_chunk(1, 1, ci)
        # AR first: its trigger + input DMA must not queue behind the
        # branch-2 table/index build
        arq1a = sb.tile([C, 2], F32, name="arq1a")
        stats_cols(stats1[1], arq1a, 0)
        art1a = allreduce(arq1a, 2, "1a")
        aff1_1 = affine_from(art1a, 0, B * N * 3, 1, (0, 1), "1a")
        build_idx9(2)
        for ci in range(NCH):
            tables_chunk(2, ci)
        conv1_init(2)
        conv2_init(1)
        for ci in range(NCH):
            conv1_chunk(2, 2, ci)
            conv2_chunk(1, aff1_1, ci)
        # merged AllReduce: branch-2 conv1 stats + branch-1 conv2 stats
        arqm = sb.tile([C, 4], F32, name="arqm")
        stats_cols(stats1[2], arqm, 0)
        stats_cols(stats2[1], arqm, 2)
        artm = allreduce(arqm, 4, "m")
        aff1_2 = affine_from(artm, 0, B * N * 3, 2, (0, 1), "1b")
        aff2_1 = affine_from(artm, 2, B * N, 1, (2, 3), "2a")
        a1, c1 = aff2_1
        conv2_init(2)
        f1_tiles = []
        for ci, (c0, cn) in enumerate(CHUNKS):
            conv2_chunk(2, aff1_2, ci)
            f1t = sb.tile([C, C], F32, name=f"f1_{ci}", tag="f1", bufs=NCH)
            nc.scalar.activation(
                out=f1t[:, :cn],
                in_=o2_tiles[1][ci][:, :cn],
                func=AF.Relu,
                scale=a1[:],
                bias=c1[:],
            )
            f1_tiles.append(f1t)
        arq2b = sb.tile([C, 2], F32, name="arq2b")
        stats_cols(stats2[2], arq2b, 0)
        art2b = allreduce(arq2b, 2, "2b")
        aff2_2 = affine_from(art2b, 0, B * N, 2, (2, 3), "2b")
        # fold delta into branch-2 affine when delta >= 0
        a2, c2 = aff2_2
        if delta_nonneg:
            a2d = sb.tile([C, 1], F32, name="a2d")
            nc.vector.tensor_tensor(out=a2d[:], in0=a2[:], in1=dcol[:], op=ALU.mult)
            c2d = sb.tile([C, 1], F32, name="c2d")
            nc.vector.tensor_tensor(out=c2d[:], in0=c2[:], in1=dcol[:], op=ALU.mult)
        for ci, (c0, cn) in enumerate(CHUNKS):
            f1t = f1_tiles[ci]
            f2t = sb.tile([C, C], F32, name=f"f2_{ci}", tag="f2", bufs=2)
            if delta_nonneg:
                nc.scalar.activation(
                    out=f2t[:, :cn],
                    in_=o2_tiles[2][ci][:, :cn],
                    func=AF.Relu,
                    scale=a2d[:],
                    bias=c2d[:],
                )
            else:
                nc.scalar.activation(
                    out=f2t[:, :cn],
                    in_=o2_tiles[2][ci][:, :cn],
                    func=AF.Relu,
                    scale=a2[:],
                    bias=c2[:],
                )
                nc.vector.tensor_scalar_mul(f2t[:, :cn], f2t[:, :cn], dcol[:])
            of = sb.tile([C, C], F32, name=f"of_{ci}", tag="of", bufs=2)
            nc.vector.tensor_tensor(
                out=of[:, :cn], in0=f1t[:, :cn], in1=f2t[:, :cn], op=ALU.add
            )
            nc.sync.dma_start(out=out_t[:, c0 : c0 + cn], in_=of[:, :cn])


# ======================= host side =======================

_CACHE = {}


def _prep_branch(w1, b1, g1, be1, w2, b2, g2, be2):
    w1 = np.asarray(w1, dtype=np.float32)
    w2 = np.asarray(w2, dtype=np.float32)
    A = w1[:, :C, 0, :]  # [o, i, 3]
    Bm = w1[:, C:, 0, :]  # [o, i, 3]
    pt = np.ascontiguousarray((A + Bm).sum(axis=2).T)  # [i, o]
    nbt = np.ascontiguousarray(
        np.concatenate([(-Bm[:, :, d]).T for d in range(3)], axis=1)
    )  # [i, 3C]
    w2t = np.ascontiguousarray(
        np.concatenate([w2[:, :, 0, d].T for d in range(3)], axis=1)
    )  # [o, 3C]
    bn = np.ascontiguousarray(
        np.stack(
            [
                np.asarray(g1, np.float32),
                np.asarray(be1, np.float32),
                np.asarray(g2, np.float32),
                np.asarray(be2, np.float32),
            ],
            axis=1,
        )
    )  # [C, 4]
    return pt, nbt, w2t, bn


def kernel(**inputs):
    features = np.ascontiguousarray(np.asarray(inputs["features"], np.float32))
    motion = np.ascontiguousarray(np.asarray(inputs["motion"], np.float32))
    delta = np.asarray(inputs["delta"], np.float32).reshape(-1)[0]

    pt1, nbt1, w2t1, bn1 = _prep_branch(
        inputs["d1_w1"], inputs["d1_b1"], inputs["d1_g1"], inputs["d1_be1"],
        inputs["d1_w2"], inputs["d1_b2"], inputs["d1_g2"], inputs["d1_be2"],
    )
    pt2, nbt2, w2t2, bn2 = _prep_branch(
        inputs["d2_w1"], inputs["d2_b1"], inputs["d2_g1"], inputs["d2_be1"],
        inputs["d2_w2"], inputs["d2_b2"], inputs["d2_g2"], inputs["d2_be2"],
    )

    delta_nonneg = bool(delta >= 0.0)
    key = ("dg", delta_nonneg)
    if key not in _CACHE:
        _CACHE[key] = build_kernel(delta_nonneg)
    nc = _CACHE[key]

    shared = {
        "pt1": pt1, "nbt1": nbt1, "w2t1": w2t1, "bn1": bn1,
        "pt2": pt2, "nbt2": nbt2, "w2t2": w2t2, "bn2": bn2,
        "delta": np.array([[delta]], np.float32),
    }
    in_maps = []
    for c in range(B):
        im = dict(shared)
        im["feat"] = np.ascontiguousarray(features[c, :, :, 0])
        im["mot"] = np.ascontiguousarray(motion[c, :, :, 0])
        in_maps.append(im)

    import os

    trace = bool(int(os.environ.get("DG_KERNEL_TRACE", "0")))
    res = bass_utils.run_bass_kernel_spmd(
        nc, in_maps, core_ids=list(range(B)), trace=trace
    )
    global LAST_RESULTS
    LAST_RESULTS = res
    out = np.stack([res.results[c]["out"] for c in range(B)], axis=0)
    return out.reshape(B, C, N, 1).astype(np.float32)


LAST_RESULTS = None

